# revision 34
# baseline (speedup 1.0000x reference)
"""CenterLoss on Trainium2 (raw Bass, SPMD over 8 NeuronCores).

Computes mean_i ||x_i - centers[label_i]||^2 (the reference clamps each
distance to [1e-12, 1e12], which never binds for this data regime).

Sharding (data-parallel over the batch, per the hint's second option:
"replicate centers"):
  - x [256, 512] and label [256] are split into 8 shards of 32 rows.
  - centers [100000, 512] is replicated to every core; each core only
    TOUCHES its 32 gathered rows (~64 KB) in HBM, so the kernel stays in
    the memory-latency regime.  Compared to vocab-sharding this cuts the
    per-core indirect-gather descriptor count from 256 to 32 (descriptor
    generation is the dominant serial cost at ~9 ns/descriptor) and
    needs no label localization, masks, bounds sentinels, or tile
    zero-fill.
  - Each core returns sum(dist_rows)/256 — its partial of the final
    mean — and the host sums the 8 partial scalars (the unshard step).

Device program per core (identical SPMD image, different data):
    lt [32, 1]   i32 <- label shard   (HWDGE, issued first)
    xt [32, 512] f32 <- x shard       (HWDGE, 64 KB)
    gt [32, 512] f32 <- ONE indirect SWDGE gather (32 descriptors),
                        enqueued at kernel start and ARMED in-queue on
                        the label-load semaphore (a DMA instruction's
                        one folded wait), so it fires the instant the
                        labels land with no sequencer round-trip.
  DVE:  gt = xt - gt
  ACT:  Square activation with scale=1/16 and accum_out:
        rs[p] = sum_d ((x-c)[p,d]/16)^2 = dist_p/256.  (The activation
        table is prefetched by a dummy square on the preamble's 0.0
        const tile, overlapping the DMA waits.)
  PE :  acc[1,1] = ones[32]^T @ rs  (ones from the preamble const pool)
  DVE:  res <- acc (PSUM -> SBUF); the HWDGE store was enqueued at
        kernel start armed on the copy's semaphore.

Raw Bass (not Tile) because this container's walrus build accepts only
one folded sync-wait per instruction ("Too many sync wait commands") and
rejects the fused DVE tensor_tensor_reduce encoding ("ISA wrong length");
standalone wait_ge instructions and BIR-native ops sidestep both.
"""

import numpy as np

import concourse.bass as bass
from concourse import mybir
from concourse.bass_utils import run_bass_kernel_spmd

NUM_CLASSES = 100000
FEAT = 512
BATCH = 256
N_CORES = 8
SHARD = BATCH // N_CORES  # 32 batch rows per core

_cache: dict = {}

# test.py reads this after calling kernel() for exec_time_ns / trace.
LAST_RESULTS = None


def _build() -> bass.Bass:
    nc = bass.Bass(enable_partition_id=False)
    x = nc.dram_tensor("x", [SHARD, FEAT], mybir.dt.float32, kind="ExternalInput")
    lab = nc.dram_tensor("lab", [SHARD], mybir.dt.int32, kind="ExternalInput")
    cen = nc.dram_tensor(
        "cen", [NUM_CLASSES, FEAT], mybir.dt.float32, kind="ExternalInput"
    )
    out = nc.dram_tensor("out", [1, 1], mybir.dt.float32, kind="ExternalOutput")

    with (
        nc.sbuf_tensor([SHARD, FEAT], mybir.dt.float32) as xt,
        nc.sbuf_tensor([SHARD, FEAT], mybir.dt.float32) as gt,
        nc.sbuf_tensor([SHARD, FEAT], mybir.dt.float32) as sq,
        nc.sbuf_tensor([SHARD, 1], mybir.dt.int32) as lt,
        nc.sbuf_tensor([SHARD, 1], mybir.dt.float32) as rs,
        nc.sbuf_tensor([1, 1], mybir.dt.float32) as res,
        nc.psum_tensor([1, 1], mybir.dt.float32) as acc,
        nc.semaphore() as s_l,    # lt load done (+16)
        nc.semaphore() as s_x,    # xt load done (+16)
        nc.semaphore() as s_g,    # gather done (+16)
        nc.semaphore() as s_v,    # DVE sub done (+1)
        nc.semaphore() as s_a,    # ACT square done (+1)
        nc.semaphore() as s_mm,   # PE matmul done (+1)
        nc.semaphore() as s_res,  # res copy done (+1)
        nc.semaphore() as s_out,  # final store done (+16)
        nc.Block() as block,
    ):

        @block.sync
        def _(sync: bass.BassEngine):
            sync.dma_start(out=lt[:], in_=lab[:, None], single_packet=True).then_inc(
                s_l, 16
            )
            sync.dma_start(out=xt[:], in_=x[:, :]).then_inc(s_x, 16)
            # Folded wait: the store is enqueued now, armed on s_res, and
            # fires the moment the result copy lands (no sequencer lag).
            sync.dma_start(out=out[:], in_=res[:])._wait_ge(s_res, 1).then_inc(
                s_out, 16
            )
            sync.wait_ge(s_out, 16)

        @block.gpsimd
        def _(gpsimd: bass.BassEngine):
            # Enqueued immediately; the queue holds it armed on the label
            # load (the one folded wait a DMA instruction may carry).
            # All labels are in [0, NUM_CLASSES) by the input contract;
            # the bounds check only guards against garbage indices.
            g = gpsimd.indirect_dma_start(
                out=gt[:],
                out_offset=None,
                in_=cen[:],
                in_offset=bass.IndirectOffsetOnAxis(ap=lt[:, :1], axis=0),
                bounds_check=NUM_CLASSES - 1,
                oob_is_err=False,
            )
            g.ins.single_packet = True
            g._wait_ge(s_l, 16).then_inc(s_g, 16)

        @block.vector
        def _(vector: bass.BassEngine):
            vector.wait_ge(s_x, 16)
            vector.wait_ge(s_g, 16)
            vector.tensor_sub(out=gt[:], in0=xt[:], in1=gt[:]).then_inc(s_v, 1)
            vector.wait_ge(s_mm, 1)
            vector.tensor_copy(out=res[:], in_=acc[:]).then_inc(s_res, 1)

        @block.scalar
        def _(scalar: bass.BassEngine):
            # Dummy square: prefetches the ACT function table while the
            # DMAs are still in flight (first ACTIVATE triggers the
            # load).  Reads the preamble's barrier-synced 0.0 const.
            scalar.square(out=res[:], in_=nc.const_aps.tensor(0.0, [1, 1]))
            scalar.wait_ge(s_v, 1)
            # rs[p] = sum_d ((x-c)/16)^2 = dist_p / 256
            scalar.activation(
                out=sq[:],
                in_=gt[:],
                func=mybir.ActivationFunctionType.Square,
                scale=1.0 / 16.0,
                accum_out=rs[:, :1],
            ).then_inc(s_a, 1)

        @block.tensor
        def _(tensor: bass.BassEngine):
            tensor.wait_ge(s_a, 1)
            tensor.matmul(
                out=acc[:],
                lhsT=nc.const_aps.tensor(1.0, [SHARD, 1]),
                rhs=rs[:, :1],
                start=True,
                stop=True,
            ).then_inc(s_mm, 1)

    return nc


def kernel(x: np.ndarray, label: np.ndarray, centers: np.ndarray) -> np.ndarray:
    global LAST_RESULTS
    x = np.ascontiguousarray(np.asarray(x, dtype=np.float32))
    centers = np.ascontiguousarray(np.asarray(centers, dtype=np.float32))
    lbl = np.asarray(label).astype(np.int64).ravel()
    assert x.shape == (BATCH, FEAT), x.shape
    assert centers.shape == (NUM_CLASSES, FEAT), centers.shape
    assert lbl.shape == (BATCH,), lbl.shape
    lbl32 = lbl.astype(np.int32)

    in_maps = []
    for i in range(N_CORES):
        sl = slice(i * SHARD, (i + 1) * SHARD)
        in_maps.append({"x": x[sl], "lab": lbl32[sl], "cen": centers})

    if "nc" not in _cache:
        _cache["nc"] = _build()
    res = run_bass_kernel_spmd(_cache["nc"], in_maps, core_ids=list(range(N_CORES)))
    LAST_RESULTS = res

    total = np.float64(0.0)
    for r in res.results:
        total += np.float64(r["out"][0, 0])
    return np.float32(total)
